# revision 1
# baseline (speedup 1.0000x reference)
"""Causal single-head attention on 8 Trainium2 NeuronCores.

Problem: x[4, 2048, 1024], Wq/Wk/Wv[1024, 1024] (torch Linear layout).
  q = x @ Wq.T ; k = x @ Wk.T ; v = x @ Wv.T
  out = softmax(mask(q @ k.T) / 32) @ v

Sharding: 8 cores = (batch b = core // 2) x (query-parity h = core % 2).
Each core computes K^T and V for the full sequence of its batch element
(duplicated across the 2 cores of a batch), plus Q^T for its own 8
query tiles (q-tiles t = 2j + h, j = 0..7), then causal attention for
those queries.  Parity interleaving makes the per-slot causal span
structure identical across cores (slot j spans 256*(j+1) keys, with the
h-dependent diagonal handled purely by per-core mask data), so a single
SPMD program serves all 8 cores.

All matmuls run as float32r (1 cycle/row on TRN2 for moving dim >= 256,
~1.5e-4 relative error vs fp32).  Host pre-transposes x and the weights
so every DMA is a wide contiguous load; softmax skips the max-subtract
(logits are O(1) after the 1/32 scale) and folds the 1/denominator into
the PSUM->SBUF eviction of the output matmul.
"""

import numpy as np

import concourse.mybir as mybir
import concourse.tile as tile
from concourse import bacc
from concourse.bass_utils import run_bass_kernel_spmd

P = 128
B = 4
S = 2048
D = 1024
ND = D // P          # d-tiles (contraction tiles for projections)
NE = D // P          # e-tiles
NQ = 8               # query slots per core (128 rows each)
SC = 512             # s-chunk: moving free dim for K/V projections
KC = 256             # k-chunk: moving free dim for scores
NCHUNK = S // SC     # 8
F32 = mybir.dt.float32
F32R = mybir.dt.float32r

MASK_VAL = -1.0e5    # additive pre-scale mask; exp((s+MASK_VAL)/32) == 0.0 in fp32

_CACHE: dict = {}


def build_program(reps: int = 1):
    """Build the single SPMD Bass program (same instruction stream on all
    8 cores; all per-core variation lives in the input data).  reps>1
    repeats the whole body serially (timing-measurement variants)."""
    nc = bacc.Bacc(None)

    xT = nc.dram_tensor("xT", [D, S], F32R, kind="ExternalInput")
    xq = nc.dram_tensor("xq", [D, NQ * P], F32R, kind="ExternalInput")
    wqT = nc.dram_tensor("wqT", [D, D], F32R, kind="ExternalInput")
    wkT = nc.dram_tensor("wkT", [D, D], F32R, kind="ExternalInput")
    wvT = nc.dram_tensor("wvT", [D, D], F32R, kind="ExternalInput")
    mask = nc.dram_tensor("mask", [NQ, P, KC], mybir.dt.bfloat16, kind="ExternalInput")
    ident = nc.dram_tensor("ident", [P, P], F32, kind="ExternalInput")
    out = nc.dram_tensor("out", [NQ * P, D], F32, kind="ExternalOutput")

    xT_r = xT[:].rearrange("(i p) s -> p i s", p=P)
    xq_r = xq[:].rearrange("(i p) q -> p i q", p=P)
    w_r = {w.name: w[:].rearrange("(i p) e -> p i e", p=P) for w in (wqT, wkT, wvT)}

    with tile.TileContext(nc) as tc:
      for _rep in range(reps):
        with (
            tc.tile_pool(name="kt", bufs=1) as ktp,
            tc.tile_pool(name="dram", bufs=1, space="DRAM") as dramp,
        ):
            # K^T resident: Kt[p, i, k] = K[k, 128i + p]
            Kt = ktp.tile([P, NE, S], F32R, tag="Kt")
            # V spilled to DRAM during the K/V phase, reloaded for attention
            v_dram = dramp.tile([S, D], F32R, tag="v_dram")
            v_dram_r = v_dram[:].rearrange("(t p) e -> p t e", p=P)

            with tc.tile_pool(name="qt", bufs=1) as qtp:
                Qt = qtp.tile([P, NE, NQ * P], F32R, tag="Qt")

                with tc.tile_pool(name="c3", bufs=1) as c3p:
                    # attention-phase constants, loaded up front
                    ident_s = c3p.tile([P, P], F32, tag="ident")
                    mask_s = c3p.tile([P, NQ, KC], mybir.dt.bfloat16, tag="mask")

                    with tc.tile_pool(name="wk1", bufs=1) as wkp:
                        wk_s = wkp.tile([P, ND, D], F32R, tag="wk")

                        def load_w(w_s, name, i):
                            nc.sync.dma_start(
                                w_s[:, i : i + 1, :], w_r[name][:, i : i + 1, :]
                            )

                        # ---- phase Q: Q^T projection ----
                        with (
                            tc.tile_pool(name="wq1", bufs=1) as wqp,
                            tc.tile_pool(name="xq2", bufs=2) as xqp,
                            tc.tile_pool(name="ps_q", bufs=2, space="PSUM") as psqp,
                        ):
                            wq_s = wqp.tile([P, ND, D], F32R, tag="wq")
                            for qc in range(NQ * P // SC):
                                xqc = xqp.tile([P, ND, SC], F32R, tag="xqc")
                                for i in range(0, ND, 2):
                                    nc.sync.dma_start(
                                        xqc[:, i : i + 2, :],
                                        xq_r[:, i : i + 2, qc * SC : (qc + 1) * SC],
                                    )
                                if qc == 0:
                                    for i in range(ND):
                                        nc.sync.dma_start(
                                            wq_s[:, i : i + 1, 0:512],
                                            w_r["wqT"][:, i : i + 1, 0:512],
                                        )
                                    for i in range(ND):
                                        nc.sync.dma_start(
                                            wq_s[:, i : i + 1, 512:D],
                                            w_r["wqT"][:, i : i + 1, 512:D],
                                        )
                                    nc.sync.dma_start(ident_s[:], ident[:])
                                    nc.sync.dma_start(
                                        mask_s[:], mask[:].rearrange("j p k -> p j k")
                                    )
                                elif qc == 1:
                                    # prefetch K-phase weights during Q compute
                                    for i in range(ND):
                                        load_w(wk_s, "wkT", i)
                                for e in range(NE):
                                    pq = psqp.tile([P, SC], F32, tag="pq")
                                    for d in range(ND):
                                        nc.tensor.matmul(
                                            pq[:],
                                            wq_s[:, d, e * P : (e + 1) * P],
                                            xqc[:, d, :],
                                            start=(d == 0),
                                            stop=(d == ND - 1),
                                        )
                                    nc.scalar.copy(
                                        Qt[:, e, qc * SC : (qc + 1) * SC], pq[:]
                                    )

                        # ---- phase KV: K^T and V projections (V -> DRAM) ----
                        with (
                            tc.tile_pool(name="wv1", bufs=1) as wvp,
                            tc.tile_pool(name="xc", bufs=2) as xcp,
                            tc.tile_pool(name="vst", bufs=1) as vstp,
                            tc.tile_pool(name="ps_k", bufs=2, space="PSUM") as pskp,
                            tc.tile_pool(name="ps_v", bufs=2, space="PSUM") as psvp,
                        ):
                            wv_s = wvp.tile([P, ND, D], F32R, tag="wv")
                            for c in range(NCHUNK):
                                xc = xcp.tile([P, ND, SC], F32R, tag="xc")
                                for i in range(0, ND, 2):
                                    nc.sync.dma_start(
                                        xc[:, i : i + 2, :],
                                        xT_r[:, i : i + 2, c * SC : (c + 1) * SC],
                                    )
                                if c == 0:
                                    for i in range(ND):
                                        load_w(wv_s, "wvT", i)
                                for e in range(NE):
                                    pk = pskp.tile([P, SC], F32, tag="pk")
                                    for d in range(ND):
                                        nc.tensor.matmul(
                                            pk[:],
                                            wk_s[:, d, e * P : (e + 1) * P],
                                            xc[:, d, :],
                                            start=(d == 0),
                                            stop=(d == ND - 1),
                                        )
                                    nc.scalar.copy(
                                        Kt[:, e, c * SC : (c + 1) * SC], pk[:]
                                    )
                                for st in range(SC // P):
                                    t_glob = c * (SC // P) + st
                                    vst = vstp.tile([P, D], F32R, tag="vst")
                                    for eh in range(2):
                                        pv = psvp.tile([P, 512], F32, tag="pv")
                                        for d in range(ND):
                                            nc.tensor.matmul(
                                                pv[:],
                                                xc[:, d, st * P : (st + 1) * P],
                                                wv_s[:, d, eh * 512 : (eh + 1) * 512],
                                                start=(d == 0),
                                                stop=(d == ND - 1),
                                            )
                                        nc.scalar.copy(
                                            vst[:, eh * 512 : (eh + 1) * 512], pv[:]
                                        )
                                    nc.sync.dma_start(
                                        v_dram[t_glob * P : (t_glob + 1) * P, :],
                                        vst[:],
                                    )

                    # ---- phase 3: attention ----
                    with (
                        tc.tile_pool(name="vv", bufs=1) as vvp,
                        tc.tile_pool(name="erow", bufs=2) as erowp,
                        tc.tile_pool(name="et", bufs=17) as etp,
                        tc.tile_pool(name="stat", bufs=2) as statp,
                        tc.tile_pool(name="orow", bufs=2) as orowp,
                        tc.tile_pool(name="ps_a", bufs=2, space="PSUM") as psap,
                        tc.tile_pool(name="ps_t", bufs=3, space="PSUM") as pstp,
                        tc.tile_pool(name="ps_s", bufs=3, space="PSUM") as pssp,
                    ):
                        Vs = vvp.tile([P, S // P, D], F32R, tag="Vs")
                        for t in range(S // P):
                            nc.sync.dma_start(
                                Vs[:, t : t + 1, :], v_dram_r[:, t : t + 1, :]
                            )

                        for j in range(NQ):
                            nk = j + 1          # 256-wide score chunks
                            nt = 2 * (j + 1)    # 128-wide key tiles
                            erow = erowp.tile([P, S], F32, tag="erow")
                            partials = statp.tile([P, NQ], F32, tag="partials")
                            den = statp.tile([P, 1], F32, tag="den")
                            rcp = statp.tile([P, 1], F32, tag="rcp")

                            # scores + exp, chunk by chunk
                            for kc in range(nk):
                                ps = pssp.tile([P, KC], F32, tag="ps")
                                for e in range(NE):
                                    nc.tensor.matmul(
                                        ps[:],
                                        Qt[:, e, j * P : (j + 1) * P],
                                        Kt[:, e, kc * KC : (kc + 1) * KC],
                                        start=(e == 0),
                                        stop=(e == NE - 1),
                                    )
                                if kc == nk - 1:
                                    # causal mask on the diagonal chunk
                                    nc.vector.tensor_add(
                                        ps[:], ps[:], mask_s[:, j, :]
                                    )
                                nc.scalar.activation(
                                    erow[:, kc * KC : (kc + 1) * KC],
                                    ps[:],
                                    mybir.ActivationFunctionType.Exp,
                                    scale=float(1.0 / np.sqrt(D)),
                                    accum_out=partials[:, kc : kc + 1],
                                )

                            # softmax denominator (no max-subtract: logits O(1))
                            nc.vector.reduce_sum(
                                den[:], partials[:, :nk], axis=mybir.AxisListType.X
                            )
                            nc.vector.reciprocal(rcp[:], den[:])

                            # transpose exp-scores; A^T.T @ V one key tile behind
                            ets = []
                            for kt in range(nt):
                                pt = pstp.tile([P, P], F32, tag="pt")
                                nc.tensor.transpose(
                                    pt[:], erow[:, kt * P : (kt + 1) * P], ident_s[:]
                                )
                                et = etp.tile([P, P], F32R, tag="et")
                                nc.vector.tensor_copy(et[:], pt[:])
                                ets.append(et)

                            orow = orowp.tile([P, D], F32, tag="orow")
                            for eh in range(2):
                                pav = psap.tile(
                                    [P, 512], F32, tag="pav", name=f"pav{j}_{eh}"
                                )
                                for kt in range(nt):
                                    nc.tensor.matmul(
                                        pav[:],
                                        ets[kt][:],
                                        Vs[:, kt, eh * 512 : (eh + 1) * 512],
                                        start=(kt == 0),
                                        stop=(kt == nt - 1),
                                    )
                                nc.vector.tensor_scalar_mul(
                                    orow[:, eh * 512 : (eh + 1) * 512],
                                    pav[:],
                                    rcp[:],
                                )
                                nc.sync.dma_start(
                                    out[j * P : (j + 1) * P, eh * 512 : (eh + 1) * 512],
                                    orow[:, eh * 512 : (eh + 1) * 512],
                                )

    nc.finalize()
    return nc


def build_program_cc(reps: int = 1):
    """K/V-split variant: each core projects K^T and V only for its own
    half of the sequence (h = core parity), then pairwise AllGather
    reconstructs the full K^T and V.  The Kt collective overlaps the V
    projection; the V collective overlaps the Q projection."""
    nc = bacc.Bacc(None)

    SH = S // 2           # local sequence half
    NCH = SH // SC        # chunks in the half (2 at SC=512)
    groups = [[0, 1], [2, 3], [4, 5], [6, 7]]

    xTh = nc.dram_tensor("xTh", [D, SH], F32R, kind="ExternalInput")
    xq = nc.dram_tensor("xq", [D, NQ * P], F32R, kind="ExternalInput")
    wqT = nc.dram_tensor("wqT", [D, D], F32R, kind="ExternalInput")
    wkT = nc.dram_tensor("wkT", [D, D], F32R, kind="ExternalInput")
    wvT = nc.dram_tensor("wvT", [D, D], F32R, kind="ExternalInput")
    mask = nc.dram_tensor("mask", [NQ, P, KC], mybir.dt.bfloat16, kind="ExternalInput")
    ident = nc.dram_tensor("ident", [P, P], F32, kind="ExternalInput")
    out = nc.dram_tensor("out", [NQ * P, D], F32, kind="ExternalOutput")

    xT_r = xTh[:].rearrange("(i p) s -> p i s", p=P)
    xq_r = xq[:].rearrange("(i p) q -> p i q", p=P)
    w_r = {w.name: w[:].rearrange("(i p) e -> p i e", p=P) for w in (wqT, wkT, wvT)}

    with tile.TileContext(nc) as tc:
      for _rep in range(reps):
        with tc.tile_pool(name="dram", bufs=1, space="DRAM") as dramp:
            kt_half = dramp.tile([D, SH], F32R, tag="kt_half")
            kt_gath = dramp.tile([2 * D, SH], F32R, tag="kt_gath")
            v_half = dramp.tile([SH, D], F32R, tag="v_half")
            v_gath = dramp.tile([S, D], F32R, tag="v_gath")
            kt_half_r = kt_half[:].rearrange("(i p) s -> p i s", p=P)
            kt_gath_r = kt_gath[:].rearrange("(h i p) s -> p h i s", h=2, p=P)
            v_half_r = v_half[:].rearrange("(t p) e -> p t e", p=P)
            v_gath_r = v_gath[:].rearrange("(t p) e -> p t e", p=P)

            with tc.tile_pool(name="qt", bufs=1) as qtp:
                Qt = qtp.tile([P, NE, NQ * P], F32R, tag="Qt")

                with tc.tile_pool(name="c3", bufs=1) as c3p:
                    ident_s = c3p.tile([P, P], F32, tag="ident")
                    mask_s = c3p.tile([P, NQ, KC], mybir.dt.bfloat16, tag="mask")

                    # ---- phase KV-half: project local K^T / V, spill, gather ----
                    with (
                        tc.tile_pool(name="w1", bufs=1) as w1p,
                        tc.tile_pool(name="xc", bufs=2) as xcp,
                        tc.tile_pool(name="kth", bufs=2) as kthp,
                        tc.tile_pool(name="vst", bufs=2) as vstp,
                        tc.tile_pool(name="ps_k", bufs=2, space="PSUM") as pskp,
                        tc.tile_pool(name="ps_v", bufs=2, space="PSUM") as psvp,
                    ):
                        wk_s = w1p.tile([P, ND, D], F32R, tag="wk")
                        wv_s = w1p.tile([P, ND, D], F32R, tag="wv")
                        xcs = []
                        for c in range(NCH):
                            xc = xcp.tile([P, ND, SC], F32R, tag="xc", name=f"xc{c}")
                            xcs.append(xc)
                        for i in range(0, ND, 2):
                            nc.sync.dma_start(
                                xcs[0][:, i : i + 2, :], xT_r[:, i : i + 2, 0:SC]
                            )
                        for i in range(ND):
                            nc.sync.dma_start(
                                wk_s[:, i : i + 1, :], w_r["wkT"][:, i : i + 1, :]
                            )
                        for i in range(0, ND, 2):
                            nc.sync.dma_start(
                                xcs[1][:, i : i + 2, :], xT_r[:, i : i + 2, SC : 2 * SC]
                            )
                        for i in range(ND):
                            nc.sync.dma_start(
                                wv_s[:, i : i + 1, :], w_r["wvT"][:, i : i + 1, :]
                            )
                        nc.sync.dma_start(ident_s[:], ident[:])
                        nc.sync.dma_start(
                            mask_s[:], mask[:].rearrange("j p k -> p j k")
                        )

                        # K^T first, so its collective overlaps V compute
                        for c in range(NCH):
                            kth = kthp.tile([P, NE, SC], F32R, tag="kth")
                            for e in range(NE):
                                pk = pskp.tile([P, SC], F32, tag="pk")
                                for d in range(ND):
                                    nc.tensor.matmul(
                                        pk[:],
                                        wk_s[:, d, e * P : (e + 1) * P],
                                        xcs[c][:, d, :],
                                        start=(d == 0),
                                        stop=(d == ND - 1),
                                    )
                                nc.scalar.copy(kth[:, e, :], pk[:])
                            for i in range(NE):
                                nc.sync.dma_start(
                                    kt_half_r[:, i : i + 1, c * SC : (c + 1) * SC],
                                    kth[:, i : i + 1, :],
                                )
                        nc.gpsimd.collective_compute(
                            "AllGather",
                            mybir.AluOpType.bypass,
                            replica_groups=groups,
                            ins=[kt_half[:]],
                            outs=[kt_gath[:]],
                        )

                        for c in range(NCH):
                            for st in range(SC // P):
                                t_loc = c * (SC // P) + st
                                vst = vstp.tile([P, D], F32R, tag="vst")
                                for eh in range(2):
                                    pv = psvp.tile([P, 512], F32, tag="pv")
                                    for d in range(ND):
                                        nc.tensor.matmul(
                                            pv[:],
                                            xcs[c][:, d, st * P : (st + 1) * P],
                                            wv_s[:, d, eh * 512 : (eh + 1) * 512],
                                            start=(d == 0),
                                            stop=(d == ND - 1),
                                        )
                                    nc.scalar.copy(
                                        vst[:, eh * 512 : (eh + 1) * 512], pv[:]
                                    )
                                nc.sync.dma_start(
                                    v_half[t_loc * P : (t_loc + 1) * P, :], vst[:]
                                )
                        nc.gpsimd.collective_compute(
                            "AllGather",
                            mybir.AluOpType.bypass,
                            replica_groups=groups,
                            ins=[v_half[:]],
                            outs=[v_gath[:]],
                        )

                    # ---- phase Q: Q^T projection (overlaps the V collective) ----
                    with (
                        tc.tile_pool(name="wq1", bufs=1) as wqp,
                        tc.tile_pool(name="xq2", bufs=2) as xqp,
                        tc.tile_pool(name="ps_q", bufs=2, space="PSUM") as psqp,
                    ):
                        wq_s = wqp.tile([P, ND, D], F32R, tag="wq")
                        for qc in range(NQ * P // SC):
                            xqc = xqp.tile([P, ND, SC], F32R, tag="xqc")
                            for i in range(0, ND, 2):
                                nc.sync.dma_start(
                                    xqc[:, i : i + 2, :],
                                    xq_r[:, i : i + 2, qc * SC : (qc + 1) * SC],
                                )
                            if qc == 0:
                                for i in range(ND):
                                    nc.sync.dma_start(
                                        wq_s[:, i : i + 1, :],
                                        w_r["wqT"][:, i : i + 1, :],
                                    )
                            for e in range(NE):
                                pq = psqp.tile([P, SC], F32, tag="pq")
                                for d in range(ND):
                                    nc.tensor.matmul(
                                        pq[:],
                                        wq_s[:, d, e * P : (e + 1) * P],
                                        xqc[:, d, :],
                                        start=(d == 0),
                                        stop=(d == ND - 1),
                                    )
                                nc.scalar.copy(
                                    Qt[:, e, qc * SC : (qc + 1) * SC], pq[:]
                                )

                    # ---- phase 3: attention ----
                    with (
                        tc.tile_pool(name="kt", bufs=1) as ktp,
                        tc.tile_pool(name="vv", bufs=1) as vvp,
                        tc.tile_pool(name="erow", bufs=2) as erowp,
                        tc.tile_pool(name="et", bufs=3) as etp,
                        tc.tile_pool(name="stat", bufs=2) as statp,
                        tc.tile_pool(name="orow", bufs=2) as orowp,
                        tc.tile_pool(name="ps_s", bufs=3, space="PSUM") as pssp,
                        tc.tile_pool(name="ps_t", bufs=2, space="PSUM") as pstp,
                        tc.tile_pool(name="ps_a", bufs=2, space="PSUM") as psap,
                    ):
                        Kt = ktp.tile([P, NE, S], F32R, tag="Kt")
                        for h2 in range(2):
                            for i in range(NE):
                                nc.sync.dma_start(
                                    Kt[:, i : i + 1, h2 * SH : (h2 + 1) * SH],
                                    kt_gath_r[:, h2, i : i + 1, :],
                                )
                        Vs = vvp.tile([P, S // P, D], F32R, tag="Vs")
                        for t in range(S // P):
                            nc.sync.dma_start(
                                Vs[:, t : t + 1, :], v_gath_r[:, t : t + 1, :]
                            )

                        for j in range(NQ):
                            nk = j + 1
                            nt = 2 * (j + 1)
                            erow = erowp.tile([P, S], F32, tag="erow")
                            partials = statp.tile([P, NQ], F32, tag="partials")
                            den = statp.tile([P, 1], F32, tag="den")
                            rcp = statp.tile([P, 1], F32, tag="rcp")

                            for kc in range(nk):
                                ps = pssp.tile([P, KC], F32, tag="ps")
                                for e in range(NE):
                                    nc.tensor.matmul(
                                        ps[:],
                                        Qt[:, e, j * P : (j + 1) * P],
                                        Kt[:, e, kc * KC : (kc + 1) * KC],
                                        start=(e == 0),
                                        stop=(e == NE - 1),
                                    )
                                if kc == nk - 1:
                                    nc.vector.tensor_add(
                                        ps[:], ps[:], mask_s[:, j, :]
                                    )
                                nc.scalar.activation(
                                    erow[:, kc * KC : (kc + 1) * KC],
                                    ps[:],
                                    mybir.ActivationFunctionType.Exp,
                                    scale=float(1.0 / np.sqrt(D)),
                                    accum_out=partials[:, kc : kc + 1],
                                )

                            nc.vector.reduce_sum(
                                den[:], partials[:, :nk], axis=mybir.AxisListType.X
                            )
                            nc.vector.reciprocal(rcp[:], den[:])

                            pavs = [
                                psap.tile([P, 512], F32, tag="pav", name=f"pav{j}_{eh}")
                                for eh in range(2)
                            ]
                            ets = []
                            for kt in range(nt):
                                pt = pstp.tile([P, P], F32, tag="pt")
                                nc.tensor.transpose(
                                    pt[:], erow[:, kt * P : (kt + 1) * P], ident_s[:]
                                )
                                et = etp.tile([P, P], F32R, tag="et")
                                nc.vector.tensor_copy(et[:], pt[:])
                                ets.append(et)
                                if kt > 0:
                                    _av_mms(nc, pavs, ets[kt - 1], Vs, kt - 1, nt)
                            _av_mms(nc, pavs, ets[nt - 1], Vs, nt - 1, nt)

                            orow = orowp.tile([P, D], F32, tag="orow")
                            for eh in range(2):
                                nc.vector.tensor_scalar_mul(
                                    orow[:, eh * 512 : (eh + 1) * 512],
                                    pavs[eh][:],
                                    rcp[:],
                                )
                            nc.sync.dma_start(out[j * P : (j + 1) * P, :], orow[:])

    nc.finalize()
    return nc


def _av_mms(nc, pavs, et, Vs, kt, nt):
    for eh in range(2):
        nc.tensor.matmul(
            pavs[eh][:],
            et[:],
            Vs[:, kt, eh * 512 : (eh + 1) * 512],
            start=(kt == 0),
            stop=(kt == nt - 1),
        )


def make_mask(h: int) -> np.ndarray:
    """Additive mask for the last 256 columns of each slot's span."""
    import ml_dtypes

    m = np.zeros((NQ, P, KC), dtype=ml_dtypes.bfloat16)
    rows = np.arange(P)[:, None]
    cols = np.arange(P)[None, :]
    tri = np.where(cols <= rows, 0.0, MASK_VAL).astype(ml_dtypes.bfloat16)
    for j in range(NQ):
        if h == 1:
            # q-tile 2j+1: first 128 cols fully valid, diagonal in last 128
            m[j, :, P:] = tri
        else:
            # q-tile 2j: diagonal in first 128 cols, last 128 fully padded
            m[j, :, :P] = tri
            m[j, :, P:] = MASK_VAL
    return m


def make_in_maps(x, Wq, Wk, Wv, cc=False):
    x = np.asarray(x, dtype=np.float32)
    wqT = np.ascontiguousarray(np.asarray(Wq, dtype=np.float32).T)
    wkT = np.ascontiguousarray(np.asarray(Wk, dtype=np.float32).T)
    wvT = np.ascontiguousarray(np.asarray(Wv, dtype=np.float32).T)
    ident = np.eye(P, dtype=np.float32)
    masks = [make_mask(0), make_mask(1)]
    in_maps = []
    for c in range(8):
        b, h = c // 2, c % 2
        xT = np.ascontiguousarray(x[b].T)                      # [D, S]
        xq = np.ascontiguousarray(
            xT.reshape(D, S // P, P)[:, [2 * j + h for j in range(NQ)], :].reshape(
                D, NQ * P
            )
        )
        entry_x = (
            {"xTh": np.ascontiguousarray(xT[:, h * (S // 2) : (h + 1) * (S // 2)])}
            if cc
            else {"xT": xT}
        )
        in_maps.append(
            {
                **entry_x,
                "xq": xq,
                "wqT": wqT,
                "wkT": wkT,
                "wvT": wvT,
                "mask": masks[h],
                "ident": ident,
            }
        )
    return in_maps


def gather_output(results) -> np.ndarray:
    out = np.empty((B, S, D), dtype=np.float32)
    for c in range(8):
        b, h = c // 2, c % 2
        oc = results[c]["out"]
        for j in range(NQ):
            t = 2 * j + h
            out[b, t * P : (t + 1) * P, :] = oc[j * P : (j + 1) * P, :]
    return out


USE_CC = False  # pairwise-AllGather K/V split: ~15% faster in the
# cost model, but repeated-collective NEFFs wedged the device once in
# testing, so the collective-free program is the default.


def kernel(x, Wq, Wk, Wv):
    key = "cc" if USE_CC else "nc"
    if key not in _CACHE:
        _CACHE[key] = build_program_cc() if USE_CC else build_program()
    nc = _CACHE[key]
    in_maps = make_in_maps(x, Wq, Wk, Wv, cc=USE_CC)
    res = run_bass_kernel_spmd(nc, in_maps, core_ids=list(range(8)))
    return gather_output(res.results)



# revision 4
# speedup vs baseline: 1.2324x; 1.2324x over previous
"""Causal single-head attention on 8 Trainium2 NeuronCores.

Problem: x[4, 2048, 1024], Wq/Wk/Wv[1024, 1024] (torch Linear layout).
  q = x @ Wq.T ; k = x @ Wk.T ; v = x @ Wv.T
  out = softmax(mask(q @ k.T) / 32) @ v

Sharding: 8 cores = (batch b = core // 2) x (query-parity h = core % 2).
Parity interleaving (q-tiles t = 2j + h) makes the per-slot causal span
structure identical across cores, so a single SPMD program serves all 8.

Algebraic restructure vs the direct form: the K and V projections of the
full sequence would be duplicated on both cores of a batch (the dominant
cost).  Instead
  scores = q @ k.T = x @ (Wq.T @ Wk) @ x.T  =: (x_q @ W_eff) @ x.T
  out    = A @ v   = (A @ x) @ Wv.T
so the full-sequence operand of both attention matmuls is the *raw
input* x (no projection), and the per-core projection work is W_eff
(one 1024^3 gemm), G = x_q @ W_eff (own queries only), and the final
(A @ x) @ Wv.T (own queries only).  Per-core tensor cycles drop from
~493k to ~353k.

All matmul operands are bf16 (1 cycle/row on TRN2 regardless of moving
width; fp32 PSUM accumulation; end-to-end rel err ~5e-3 vs the 2e-2
gate).  Softmax skips the max-subtract (logits are O(1) after the 1/32
scale); 1/denominator is folded into the PSUM->SBUF eviction of A@x.
"""

import numpy as np

import concourse.mybir as mybir
import concourse.tile as tile
from concourse import bacc
from concourse.bass_utils import run_bass_kernel_spmd

P = 128
B = 4
S = 2048
D = 1024
ND = D // P          # 128-chunks along any d/e/f/g axis (8)
NQ = 8               # query slots per core (128 rows each)
KC = 256             # key-chunk width for the score matmuls
NT = S // P          # 128-row key tiles in the full sequence (16)
F32 = mybir.dt.float32
BF16 = mybir.dt.bfloat16

MASK_VAL = -1.0e5    # additive pre-scale mask; exp((s+MASK_VAL)/32) == 0.0

_CACHE: dict = {}


def build_program(reps: int = 1):
    """Single SPMD Bass program (same instruction stream on all 8 cores;
    per-core variation lives in the input data).  reps>1 repeats the
    body serially (timing-measurement variants)."""
    nc = bacc.Bacc(None)

    wq = nc.dram_tensor("wq", [D, D], BF16, kind="ExternalInput")
    wk = nc.dram_tensor("wk", [D, D], BF16, kind="ExternalInput")
    wvT = nc.dram_tensor("wvT", [D, D], BF16, kind="ExternalInput")
    xq = nc.dram_tensor("xq", [D, NQ * P], BF16, kind="ExternalInput")
    xT = nc.dram_tensor("xT", [D, S], BF16, kind="ExternalInput")
    xn = nc.dram_tensor("xn", [S, D], BF16, kind="ExternalInput")
    mask = nc.dram_tensor("mask", [NQ, P, KC], BF16, kind="ExternalInput")
    ident = nc.dram_tensor("ident", [P, P], BF16, kind="ExternalInput")
    out = nc.dram_tensor("out", [NQ * P, D], F32, kind="ExternalOutput")

    wq_r = wq[:].rearrange("(i p) f -> p i f", p=P)
    wk_r = wk[:].rearrange("(i p) g -> p i g", p=P)
    wvT_r = wvT[:].rearrange("(i p) e -> p i e", p=P)
    xq_r = xq[:].rearrange("(i p) q -> p i q", p=P)
    xT_r = xT[:].rearrange("(i p) k -> p i k", p=P)
    xn_r = xn[:].rearrange("(t p) d -> p t d", p=P)

    with tile.TileContext(nc) as tc:
      for _rep in range(reps):
        with tc.tile_pool(name="big", bufs=1) as bigp:
            xT_s = bigp.tile([P, ND, S], BF16, tag="xT")
            xn_s = bigp.tile([P, NT, D], BF16, tag="xn")
            wvT_s = bigp.tile([P, ND, D], BF16, tag="wvT")
            G_s = bigp.tile([P, ND, NQ * P], BF16, tag="G")
            mask_s = bigp.tile([P, NQ, KC], BF16, tag="mask")
            ident_s = bigp.tile([P, P], BF16, tag="ident")

            # ---- phase W/G: W_eff = Wq^T Wk, then G^T = W_eff^T x_q^T ----
            with (
                tc.tile_pool(name="wph", bufs=1) as wp,
                tc.tile_pool(name="ps_w", bufs=2, space="PSUM") as pswp,
            ):
                wq_s = wp.tile([P, ND, D], BF16, tag="wq")
                wk_s = wp.tile([P, ND, D], BF16, tag="wk")
                W_s = wp.tile([P, ND, D], BF16, tag="W")
                xq_s = wp.tile([P, ND, NQ * P], BF16, tag="xq")

                # critical loads first; everything else overlaps compute
                for i in range(0, ND, 2):
                    nc.sync.dma_start(wq_s[:, i : i + 2, :], wq_r[:, i : i + 2, :])
                for i in range(0, ND, 2):
                    nc.sync.dma_start(wk_s[:, i : i + 2, :], wk_r[:, i : i + 2, :])
                for i in range(0, ND, 2):
                    nc.sync.dma_start(xq_s[:, i : i + 2, :], xq_r[:, i : i + 2, :])
                for i in range(0, ND, 2):
                    nc.sync.dma_start(xT_s[:, i : i + 2, :], xT_r[:, i : i + 2, :])
                for t in range(0, NT, 4):
                    nc.sync.dma_start(xn_s[:, t : t + 4, :], xn_r[:, t : t + 4, :])
                for i in range(0, ND, 2):
                    nc.sync.dma_start(wvT_s[:, i : i + 2, :], wvT_r[:, i : i + 2, :])
                nc.sync.dma_start(mask_s[:], mask[:].rearrange("j p k -> p j k"))
                nc.sync.dma_start(ident_s[:], ident[:])

                # W_eff[f, g] = sum_e Wq[e, f] Wk[e, g]   (f in partitions)
                for fc in range(ND):
                    for gh in range(2):
                        pw = pswp.tile([P, 512], F32, tag="pw")
                        for ec in range(ND):
                            nc.tensor.matmul(
                                pw[:],
                                wq_s[:, ec, fc * P : (fc + 1) * P],
                                wk_s[:, ec, gh * 512 : (gh + 1) * 512],
                                start=(ec == 0),
                                stop=(ec == ND - 1),
                            )
                        nc.scalar.copy(W_s[:, fc, gh * 512 : (gh + 1) * 512], pw[:])

                # G^T[g, q] = sum_f W_eff[f, g] x_q^T[f, q]  (g in partitions)
                for gc in range(ND):
                    for qh in range(2):
                        pg = pswp.tile([P, 512], F32, tag="pg")
                        for fc in range(ND):
                            nc.tensor.matmul(
                                pg[:],
                                W_s[:, fc, gc * P : (gc + 1) * P],
                                xq_s[:, fc, qh * 512 : (qh + 1) * 512],
                                start=(fc == 0),
                                stop=(fc == ND - 1),
                            )
                        nc.scalar.copy(G_s[:, gc, qh * 512 : (qh + 1) * 512], pg[:])

            # ---- phase A: attention + output projection ----
            with (
                tc.tile_pool(name="erow", bufs=2) as erowp,
                tc.tile_pool(name="et", bufs=3) as etp,
                tc.tile_pool(name="stat", bufs=2) as statp,
                tc.tile_pool(name="ax", bufs=2) as axp,
                tc.tile_pool(name="axt", bufs=2) as axtp,
                tc.tile_pool(name="orow", bufs=2) as orowp,
                tc.tile_pool(name="ps_s", bufs=3, space="PSUM") as pssp,
                tc.tile_pool(name="ps_t", bufs=2, space="PSUM") as pstp,
                tc.tile_pool(name="ps_a", bufs=2, space="PSUM") as psap,
            ):

                def axt_proj(ax, j):
                    # AX^T via transposes, then out = (AX) @ Wv^T
                    axt = axtp.tile([P, ND, P], BF16, tag="axt")
                    for dc in range(ND):
                        pt = pstp.tile([P, P], BF16, tag="pt")
                        nc.tensor.transpose(
                            pt[:], ax[:, dc * P : (dc + 1) * P], ident_s[:]
                        )
                        nc.vector.tensor_copy(axt[:, dc, :], pt[:])
                    orow = orowp.tile([P, D], F32, tag="orow")
                    for eh in range(2):
                        po = psap.tile([P, 512], F32, tag="pav", name=f"po{j}_{eh}")
                        for dc in range(ND):
                            nc.tensor.matmul(
                                po[:],
                                axt[:, dc, :],
                                wvT_s[:, dc, eh * 512 : (eh + 1) * 512],
                                start=(dc == 0),
                                stop=(dc == ND - 1),
                            )
                        nc.scalar.copy(orow[:, eh * 512 : (eh + 1) * 512], po[:])
                    nc.sync.dma_start(out[j * P : (j + 1) * P, :], orow[:])

                pending = None
                for j in range(NQ):
                    nk = j + 1          # 256-wide score chunks
                    nt = 2 * (j + 1)    # 128-wide key tiles
                    erow = erowp.tile([P, S], BF16, tag="erow")
                    partials = statp.tile([P, NQ], F32, tag="partials")
                    den = statp.tile([P, 1], F32, tag="den")
                    rcp = statp.tile([P, 1], F32, tag="rcp")

                    # scores = G @ x^T, chunk by chunk; exp into erow
                    for kc in range(nk):
                        ps = pssp.tile([P, KC], F32, tag="ps")
                        for gc in range(ND):
                            nc.tensor.matmul(
                                ps[:],
                                G_s[:, gc, j * P : (j + 1) * P],
                                xT_s[:, gc, kc * KC : (kc + 1) * KC],
                                start=(gc == 0),
                                stop=(gc == ND - 1),
                            )
                        if kc == nk - 1:
                            # causal mask on the diagonal chunk
                            nc.vector.tensor_add(ps[:], ps[:], mask_s[:, j, :])
                        nc.scalar.activation(
                            erow[:, kc * KC : (kc + 1) * KC],
                            ps[:],
                            mybir.ActivationFunctionType.Exp,
                            scale=float(1.0 / np.sqrt(D)),
                            accum_out=partials[:, kc : kc + 1],
                        )

                    nc.vector.reduce_sum(
                        den[:], partials[:, :nk], axis=mybir.AxisListType.X
                    )
                    nc.vector.reciprocal(rcp[:], den[:])

                    # previous slot's output projection fills the gap while
                    # this slot's exps drain on the scalar engine
                    if pending is not None:
                        axt_proj(*pending)

                    # transpose exp-scores; AX = A^T.T @ x one key tile behind
                    paxs = [
                        psap.tile([P, 512], F32, tag="pav", name=f"pax{j}_{eh}")
                        for eh in range(2)
                    ]
                    ets = []
                    for kt in range(nt):
                        pt = pstp.tile([P, P], BF16, tag="pt")
                        nc.tensor.transpose(
                            pt[:], erow[:, kt * P : (kt + 1) * P], ident_s[:]
                        )
                        et = etp.tile([P, P], BF16, tag="et")
                        nc.vector.tensor_copy(et[:], pt[:])
                        ets.append(et)
                        if kt > 0:
                            _ax_mms(nc, paxs, ets[kt - 1], xn_s, kt - 1, nt)
                    _ax_mms(nc, paxs, ets[nt - 1], xn_s, nt - 1, nt)

                    # AX eviction folds in the softmax 1/denominator
                    ax = axp.tile([P, D], BF16, tag="ax")
                    for eh in range(2):
                        nc.vector.tensor_scalar_mul(
                            ax[:, eh * 512 : (eh + 1) * 512], paxs[eh][:], rcp[:]
                        )
                    pending = (ax, j)

                axt_proj(*pending)

    nc.finalize()
    return nc


def _ax_mms(nc, paxs, et, xn_s, kt, nt):
    for eh in range(2):
        nc.tensor.matmul(
            paxs[eh][:],
            et[:],
            xn_s[:, kt, eh * 512 : (eh + 1) * 512],
            start=(kt == 0),
            stop=(kt == nt - 1),
        )


def make_mask(h: int) -> np.ndarray:
    """Additive mask for the last 256 columns of each slot's span."""
    import ml_dtypes

    m = np.zeros((NQ, P, KC), dtype=ml_dtypes.bfloat16)
    rows = np.arange(P)[:, None]
    cols = np.arange(P)[None, :]
    tri = np.where(cols <= rows, 0.0, MASK_VAL).astype(ml_dtypes.bfloat16)
    for j in range(NQ):
        if h == 1:
            # q-tile 2j+1: first 128 cols fully valid, diagonal in last 128
            m[j, :, P:] = tri
        else:
            # q-tile 2j: diagonal in first 128 cols, last 128 fully padded
            m[j, :, :P] = tri
            m[j, :, P:] = MASK_VAL
    return m


def make_in_maps(x, Wq, Wk, Wv):
    import ml_dtypes

    bf16 = ml_dtypes.bfloat16
    x = np.asarray(x, dtype=np.float32)
    wq_b = np.ascontiguousarray(np.asarray(Wq, dtype=np.float32).astype(bf16))
    wk_b = np.ascontiguousarray(np.asarray(Wk, dtype=np.float32).astype(bf16))
    wvT_b = np.ascontiguousarray(np.asarray(Wv, dtype=np.float32).T.astype(bf16))
    ident = np.eye(P, dtype=bf16)
    masks = [make_mask(0), make_mask(1)]
    in_maps = []
    for c in range(8):
        b, h = c // 2, c % 2
        xb = x[b].astype(bf16)                                  # [S, D]
        xT_b = np.ascontiguousarray(xb.T)                       # [D, S]
        xq_b = np.ascontiguousarray(
            xT_b.reshape(D, NT, P)[:, [2 * j + h for j in range(NQ)], :].reshape(
                D, NQ * P
            )
        )
        in_maps.append(
            {
                "wq": wq_b,
                "wk": wk_b,
                "wvT": wvT_b,
                "xq": xq_b,
                "xT": xT_b,
                "xn": xb,
                "mask": masks[h],
                "ident": ident,
            }
        )
    return in_maps


def gather_output(results) -> np.ndarray:
    out = np.empty((B, S, D), dtype=np.float32)
    for c in range(8):
        b, h = c // 2, c % 2
        oc = results[c]["out"]
        for j in range(NQ):
            t = 2 * j + h
            out[b, t * P : (t + 1) * P, :] = oc[j * P : (j + 1) * P, :]
    return out


def kernel(x, Wq, Wk, Wv):
    if "p1" not in _CACHE:
        _CACHE["p1"] = build_program()
    nc = _CACHE["p1"]
    in_maps = make_in_maps(x, Wq, Wk, Wv)
    res = run_bass_kernel_spmd(nc, in_maps, core_ids=list(range(8)))
    return gather_output(res.results)


# revision 12
# speedup vs baseline: 1.3860x; 1.1247x over previous
"""Causal single-head attention on 8 Trainium2 NeuronCores.

Problem: x[4, 2048, 1024], Wq/Wk/Wv[1024, 1024] (torch Linear layout).
  q = x @ Wq.T ; k = x @ Wk.T ; v = x @ Wv.T
  out = softmax(mask(q @ k.T) / 32) @ v

Sharding: 8 cores = (batch b = core // 2) x (query-parity h = core % 2).
Parity interleaving (q-tiles t = 2j + h) makes the per-slot causal span
structure identical across cores, so a single SPMD program serves all 8.

Algebraic restructure vs the direct form: the K and V projections of the
full sequence would be duplicated on both cores of a batch (the dominant
cost).  Instead
  scores = q @ k.T = x @ (Wq.T @ Wk) @ x.T  =: (x_q @ W_eff) @ x.T
  out    = A @ v   = (A @ x) @ Wv.T
so the full-sequence operand of both attention matmuls is the *raw
input* x (no projection), and the per-core projection work is W_eff
(one 1024^3 gemm), G = x_q @ W_eff (own queries only), and the final
(A @ x) @ Wv.T (own queries only).  Per-core tensor cycles drop from
~493k to ~345k.

All matmul operands are bf16 (1 cycle/row on TRN2 regardless of moving
width; fp32 PSUM accumulation; end-to-end rel err ~4e-3 vs the 2e-2
gate).  A@x is accumulated directly in transposed [d, q] layout (x key
tiles stationary, transposed exp-scores moving), so no second transpose
pass is needed before the output projection; softmax skips the
max-subtract and 1/denominator is folded into the final PSUM->SBUF
eviction of the output row.

Scheduling notes (PE bubbles cost double: the clock drops to 1.2 GHz
for 3 us after any idle gap):
  - W_eff starts as 7 concurrent PSUM chains stepped by contraction
    chunk, so compute starts as soon as the first wq/wk slices land
    instead of waiting for the full weight load; wk loads its g-halves
    separately since the first chains only read columns 0:512.
  - Slot 0's score chunk is computed between the last two G chains
    (using the 8th PSUM bank) so its exp hides under G compute and the
    PSUM pool transition.
  - Each slot's output projection is emitted *between* the score chunks
    of the next slot, keeping PE fed while the scalar engine drains
    exps and evictions.
"""

import numpy as np

import concourse.mybir as mybir
import concourse.tile as tile
from concourse import bacc
from concourse.bass_utils import run_bass_kernel_spmd

P = 128
B = 4
S = 2048
D = 1024
ND = D // P          # 128-chunks along any d/e/f/g axis (8)
NQ = 8               # query slots per core (128 rows each)
KC = 256             # key-chunk width for the score matmuls
NT = S // P          # 128-row key tiles in the full sequence (16)
F32 = mybir.dt.float32
BF16 = mybir.dt.bfloat16

MASK_VAL = -1.0e5    # additive pre-scale mask; exp((s+MASK_VAL)/32) == 0.0

_CACHE: dict = {}


def build_program(reps: int = 1):
    """Single SPMD Bass program (same instruction stream on all 8 cores;
    per-core variation lives in the input data).  reps>1 repeats the
    body serially (timing-measurement variants)."""
    nc = bacc.Bacc(None)

    wq = nc.dram_tensor("wq", [D, D], BF16, kind="ExternalInput")
    wk = nc.dram_tensor("wk", [D, D], BF16, kind="ExternalInput")
    wvT = nc.dram_tensor("wvT", [D, D], BF16, kind="ExternalInput")
    xq = nc.dram_tensor("xq", [D, NQ * P], BF16, kind="ExternalInput")
    xT = nc.dram_tensor("xT", [D, S], BF16, kind="ExternalInput")
    xn = nc.dram_tensor("xn", [S, D], BF16, kind="ExternalInput")
    mask = nc.dram_tensor("mask", [NQ, P, KC], BF16, kind="ExternalInput")
    ident = nc.dram_tensor("ident", [P, P], BF16, kind="ExternalInput")
    out = nc.dram_tensor("out", [NQ * P, D], F32, kind="ExternalOutput")

    wq_r = wq[:].rearrange("(i p) f -> p i f", p=P)
    wk_r = wk[:].rearrange("(i p) g -> p i g", p=P)
    wvT_r = wvT[:].rearrange("(i p) e -> p i e", p=P)
    xq_r = xq[:].rearrange("(i p) q -> p i q", p=P)
    xT_r = xT[:].rearrange("(i p) k -> p i k", p=P)
    xn_r = xn[:].rearrange("(t p) d -> p t d", p=P)

    with tile.TileContext(nc) as tc:
      for _rep in range(reps):
        with (
            tc.tile_pool(name="big", bufs=1) as bigp,
            tc.tile_pool(name="erow", bufs=2) as erowp,
            tc.tile_pool(name="et", bufs=17) as etp,
            tc.tile_pool(name="stat", bufs=2) as statp,
            tc.tile_pool(name="axt", bufs=2) as axtp,
            tc.tile_pool(name="orow", bufs=2) as orowp,
        ):
            xT_s = bigp.tile([P, ND, S], BF16, tag="xT")
            xn_s = bigp.tile([P, NT, D], BF16, tag="xn")
            wvT_s = bigp.tile([P, ND, D], BF16, tag="wvT")
            G_s = bigp.tile([P, ND, NQ * P], BF16, tag="G")
            mask_s = bigp.tile([P, NQ, KC], BF16, tag="mask")
            ident_s = bigp.tile([P, P], BF16, tag="ident")

            def score_chunk(psp, j, kc, nk, erow, partials, tag="ps", bufs=None):
                ps = psp.tile([P, KC], F32, tag=tag, bufs=bufs)
                for gc in range(ND):
                    nc.tensor.matmul(
                        ps[:],
                        G_s[:, gc, j * P : (j + 1) * P],
                        xT_s[:, gc, kc * KC : (kc + 1) * KC],
                        start=(gc == 0),
                        stop=(gc == ND - 1),
                    )
                if kc == nk - 1:
                    # causal mask on the diagonal chunk
                    nc.vector.tensor_add(ps[:], ps[:], mask_s[:, j, :])
                nc.scalar.activation(
                    erow[:, kc * KC : (kc + 1) * KC],
                    ps[:],
                    mybir.ActivationFunctionType.Exp,
                    scale=float(1.0 / np.sqrt(D)),
                    accum_out=partials[:, kc : kc + 1],
                )

            erow0 = erowp.tile([P, S], BF16, tag="erow", name="erow0")
            partials0 = statp.tile([P, NQ], F32, tag="partials", name="partials0")
            den0 = statp.tile([P, 1], F32, tag="den", name="den0")
            rcp0 = statp.tile([P, 1], F32, tag="rcp", name="rcp0")

            # ---- phase W/G: W_eff = Wq^T Wk, then G^T = W_eff^T x_q^T ----
            with (
                tc.tile_pool(name="wph", bufs=1) as wp,
                tc.tile_pool(name="ps_w", bufs=7, space="PSUM") as pswp,
            ):
                wq_s = wp.tile([P, ND, D], BF16, tag="wq")
                wk_s = wp.tile([P, ND, D], BF16, tag="wk")
                W_s = wp.tile([P, ND, D], BF16, tag="W")
                xq_s = wp.tile([P, ND, NQ * P], BF16, tag="xq")

                # wq/wk interleaved at fine granularity so the ec-stepped
                # W_eff chains below start as soon as the first slices
                # land; wk split by g-half (first chains read cols 0:512)
                for i in range(2):
                    nc.sync.dma_start(wq_s[:, i : i + 1, :], wq_r[:, i : i + 1, :])
                    nc.sync.dma_start(
                        wk_s[:, i : i + 1, 0:512], wk_r[:, i : i + 1, 0:512]
                    )
                for i in range(2, ND, 2):
                    nc.sync.dma_start(wq_s[:, i : i + 2, :], wq_r[:, i : i + 2, :])
                    nc.sync.dma_start(
                        wk_s[:, i : i + 2, 0:512], wk_r[:, i : i + 2, 0:512]
                    )
                for i in range(0, ND, 2):
                    nc.sync.dma_start(
                        wk_s[:, i : i + 2, 512:D], wk_r[:, i : i + 2, 512:D]
                    )
                for i in range(0, ND, 2):
                    nc.sync.dma_start(xq_s[:, i : i + 2, :], xq_r[:, i : i + 2, :])
                for i in range(0, ND, 2):
                    nc.sync.dma_start(xT_s[:, i : i + 2, :], xT_r[:, i : i + 2, :])
                for t in range(0, NT, 4):
                    nc.sync.dma_start(xn_s[:, t : t + 4, :], xn_r[:, t : t + 4, :])
                for i in range(0, ND, 2):
                    nc.sync.dma_start(wvT_s[:, i : i + 2, :], wvT_r[:, i : i + 2, :])
                nc.sync.dma_start(mask_s[:], mask[:].rearrange("j p k -> p j k"))
                nc.sync.dma_start(ident_s[:], ident[:])

                # W_eff[f, g] = sum_e Wq[e, f] Wk[e, g]   (f in partitions)
                def w_chain(gh, fc, pw):
                    for ec in range(ND):
                        nc.tensor.matmul(
                            pw[:],
                            wq_s[:, ec, fc * P : (fc + 1) * P],
                            wk_s[:, ec, gh * 512 : (gh + 1) * 512],
                            start=(ec == 0),
                            stop=(ec == ND - 1),
                        )

                # window of 7 ec-stepped chains overlapping the weight DMA
                pws = [
                    pswp.tile([P, 512], F32, tag="pw", name=f"pw0_{fc}")
                    for fc in range(7)
                ]
                for ec in range(ND):
                    for fc in range(7):
                        nc.tensor.matmul(
                            pws[fc][:],
                            wq_s[:, ec, fc * P : (fc + 1) * P],
                            wk_s[:, ec, 0:512],
                            start=(ec == 0),
                            stop=(ec == ND - 1),
                        )
                for fc in range(7):
                    nc.scalar.copy(W_s[:, fc, 0:512], pws[fc][:])
                for gh, fc in [(0, 7)] + [(1, fc) for fc in range(ND)]:
                    pw = pswp.tile([P, 512], F32, tag="pw", name=f"pw{gh}_{fc}")
                    w_chain(gh, fc, pw)
                    nc.scalar.copy(W_s[:, fc, gh * 512 : (gh + 1) * 512], pw[:])

                # G^T[g, q] = sum_f W_eff[f, g] x_q^T[f, q]  (g in partitions)
                # qh-outer: scores of slot 0 need q-columns 0:128 for all gc
                def g_chain(qh, gc):
                    pg = pswp.tile([P, 512], F32, tag="pw", name=f"pg{qh}_{gc}")
                    for fc in range(ND):
                        nc.tensor.matmul(
                            pg[:],
                            W_s[:, fc, gc * P : (gc + 1) * P],
                            xq_s[:, fc, qh * 512 : (qh + 1) * 512],
                            start=(fc == 0),
                            stop=(fc == ND - 1),
                        )
                    nc.scalar.copy(G_s[:, gc, qh * 512 : (qh + 1) * 512], pg[:])

                for gc in range(ND):
                    g_chain(0, gc)
                for gc in range(ND - 1):
                    g_chain(1, gc)
                # slot 0's single score chunk, on the spare PSUM bank; its
                # mask/exp drain while the last G chain computes
                score_chunk(pswp, 0, 0, 1, erow0, partials0, tag="ps0", bufs=1)
                nc.vector.reduce_sum(
                    den0[:], partials0[:, 0:1], axis=mybir.AxisListType.X
                )
                nc.vector.reciprocal(rcp0[:], den0[:])
                g_chain(1, ND - 1)

            # ---- phase A: attention + output projection ----
            with (
                tc.tile_pool(name="ps_s", bufs=3, space="PSUM") as pssp,
                tc.tile_pool(name="ps_t", bufs=2, space="PSUM") as pstp,
                tc.tile_pool(name="ps_a", bufs=2, space="PSUM") as psap,
            ):

                def proj_flush(axt, rcp, j):
                    # out = (AX) @ Wv^T, normalized by 1/den at eviction
                    orow = orowp.tile([P, D], F32, tag="orow")
                    for eh in range(2):
                        po = psap.tile([P, 512], F32, tag="pav", name=f"po{j}_{eh}")
                        for dc in range(ND):
                            nc.tensor.matmul(
                                po[:],
                                axt[:, dc * P : (dc + 1) * P],
                                wvT_s[:, dc, eh * 512 : (eh + 1) * 512],
                                start=(dc == 0),
                                stop=(dc == ND - 1),
                            )
                        nc.vector.tensor_scalar_mul(
                            orow[:, eh * 512 : (eh + 1) * 512], po[:], rcp[:]
                        )
                        nc.sync.dma_start(
                            out[j * P : (j + 1) * P, eh * 512 : (eh + 1) * 512],
                            orow[:, eh * 512 : (eh + 1) * 512],
                        )

                def axt_group(paxs, ets, dc, nt):
                    # AX^T[d, q] for one 128-wide d-chunk: x key tiles
                    # stationary, transposed exp-scores moving.  One PSUM
                    # accumulation group at a time per bank (the zero
                    # region is the bank, groups must not interleave).
                    sub = dc % 4
                    for kt in range(nt):
                        nc.tensor.matmul(
                            paxs[dc // 4][:, sub * P : (sub + 1) * P],
                            xn_s[:, kt, dc * P : (dc + 1) * P],
                            ets[kt][:],
                            start=(kt == 0),
                            stop=(kt == nt - 1),
                        )

                pending = None
                for j in range(NQ):
                    nk = j + 1          # 256-wide score chunks
                    nt = 2 * (j + 1)    # 128-wide key tiles
                    if j == 0:
                        erow, partials, den, rcp = erow0, partials0, den0, rcp0
                    else:
                        erow = erowp.tile([P, S], BF16, tag="erow")
                        partials = statp.tile([P, NQ], F32, tag="partials")
                        den = statp.tile([P, 1], F32, tag="den")
                        rcp = statp.tile([P, 1], F32, tag="rcp")

                        for kc in range(nk - 1):
                            score_chunk(pssp, j, kc, nk, erow, partials)
                        if pending is not None:
                            proj_flush(*pending)
                        score_chunk(pssp, j, nk - 1, nk, erow, partials)

                        nc.vector.reduce_sum(
                            den[:], partials[:, :nk], axis=mybir.AxisListType.X
                        )
                        nc.vector.reciprocal(rcp[:], den[:])

                    # transpose exp-scores, then accumulate AX^T one
                    # 128-wide d-chunk (= one PSUM group) at a time
                    paxs = [
                        psap.tile([P, 512], F32, tag="pav", name=f"pax{j}_{dh}")
                        for dh in range(2)
                    ]
                    ets = []
                    for kt in range(nt):
                        pt = pstp.tile([P, P], BF16, tag="pt")
                        nc.tensor.transpose(
                            pt[:], erow[:, kt * P : (kt + 1) * P], ident_s[:]
                        )
                        et = etp.tile([P, P], BF16, tag="et")
                        nc.vector.tensor_copy(et[:], pt[:])
                        ets.append(et)
                    axt = axtp.tile([P, D], BF16, tag="axt")
                    for dc in range(ND):
                        axt_group(paxs, ets, dc, nt)
                        if dc % 4 == 3:
                            dh = dc // 4
                            nc.scalar.copy(
                                axt[:, dh * 512 : (dh + 1) * 512], paxs[dh][:]
                            )
                    pending = (axt, rcp, j)

                proj_flush(*pending)

    nc.finalize()
    return nc


def make_mask(h: int) -> np.ndarray:
    """Additive mask for the last 256 columns of each slot's span."""
    import ml_dtypes

    m = np.zeros((NQ, P, KC), dtype=ml_dtypes.bfloat16)
    rows = np.arange(P)[:, None]
    cols = np.arange(P)[None, :]
    tri = np.where(cols <= rows, 0.0, MASK_VAL).astype(ml_dtypes.bfloat16)
    for j in range(NQ):
        if h == 1:
            # q-tile 2j+1: first 128 cols fully valid, diagonal in last 128
            m[j, :, P:] = tri
        else:
            # q-tile 2j: diagonal in first 128 cols, last 128 fully padded
            m[j, :, :P] = tri
            m[j, :, P:] = MASK_VAL
    return m


def make_in_maps(x, Wq, Wk, Wv):
    import ml_dtypes

    bf16 = ml_dtypes.bfloat16
    x = np.asarray(x, dtype=np.float32)
    wq_b = np.ascontiguousarray(np.asarray(Wq, dtype=np.float32).astype(bf16))
    wk_b = np.ascontiguousarray(np.asarray(Wk, dtype=np.float32).astype(bf16))
    wvT_b = np.ascontiguousarray(np.asarray(Wv, dtype=np.float32).T.astype(bf16))
    ident = np.eye(P, dtype=bf16)
    masks = [make_mask(0), make_mask(1)]
    in_maps = []
    for c in range(8):
        b, h = c // 2, c % 2
        xb = x[b].astype(bf16)                                  # [S, D]
        xT_b = np.ascontiguousarray(xb.T)                       # [D, S]
        xq_b = np.ascontiguousarray(
            xT_b.reshape(D, NT, P)[:, [2 * j + h for j in range(NQ)], :].reshape(
                D, NQ * P
            )
        )
        in_maps.append(
            {
                "wq": wq_b,
                "wk": wk_b,
                "wvT": wvT_b,
                "xq": xq_b,
                "xT": xT_b,
                "xn": xb,
                "mask": masks[h],
                "ident": ident,
            }
        )
    return in_maps


def gather_output(results) -> np.ndarray:
    out = np.empty((B, S, D), dtype=np.float32)
    for c in range(8):
        b, h = c // 2, c % 2
        oc = results[c]["out"]
        for j in range(NQ):
            t = 2 * j + h
            out[b, t * P : (t + 1) * P, :] = oc[j * P : (j + 1) * P, :]
    return out


def kernel(x, Wq, Wk, Wv):
    if "p1" not in _CACHE:
        _CACHE["p1"] = build_program()
    nc = _CACHE["p1"]
    in_maps = make_in_maps(x, Wq, Wk, Wv)
    res = run_bass_kernel_spmd(nc, in_maps, core_ids=list(range(8)))
    return gather_output(res.results)


# revision 14
# speedup vs baseline: 1.4862x; 1.0723x over previous
"""Causal single-head attention on 8 Trainium2 NeuronCores.

Problem: x[4, 2048, 1024], Wq/Wk/Wv[1024, 1024] (torch Linear layout).
  q = x @ Wq.T ; k = x @ Wk.T ; v = x @ Wv.T
  out = softmax(mask(q @ k.T) / 32) @ v

Sharding: 8 cores = (batch b = core // 2) x (query-parity h = core % 2).
Parity interleaving (q-tiles t = 2j + h) makes the per-slot causal span
structure identical across cores, so a single SPMD program serves all 8.

Algebraic restructure vs the direct form: the K and V projections of the
full sequence would be duplicated on both cores of a batch (the dominant
cost).  Instead
  scores = q @ k.T = x @ (Wq.T @ Wk) @ x.T  =: (x_q @ W_eff) @ x.T
  out    = A @ v   = (A @ x) @ Wv.T
so the full-sequence operand of both attention matmuls is the *raw
input* x (no projection), and the per-core projection work is W_eff
(one 1024^3 gemm), G = x_q @ W_eff (own queries only), and the final
(A @ x) @ Wv.T (own queries only).  Per-core tensor cycles drop from
~493k to ~344k.

All matmul operands are bf16 (1 cycle/row on TRN2 regardless of moving
width; fp32 PSUM accumulation; end-to-end rel err ~4e-3 vs the 2e-2
gate).  Scores are computed directly transposed ([key, query] tiles,
x^T tiles stationary / G tiles moving), so exp writes the A^T operand
of the A@x matmul in place -- no transpose pass.  The softmax
denominator comes from near-free 1-column matmuls den = A^T.T @ ones;
the max-subtract is skipped (logits are O(1) after the 1/32 scale) and
1/den is folded into the final eviction of the output row.

Scheduling notes (PE bubbles cost double: the clock drops to 1.2 GHz
for 3 us after any idle gap):
  - W_eff starts as 6 concurrent PSUM chains stepped by contraction
    chunk, so compute starts as soon as the first wq/wk slices land
    instead of waiting for the full weight load; wk loads its g-halves
    separately since the first chains only read columns 0:512.
  - Slot 0's score tiles are computed between the last two G chains
    (on spare PSUM banks) so their exps hide under G compute and the
    PSUM pool transition.
  - Each slot's output projection is emitted *between* the score tiles
    and the A@x accumulation of the next slot, keeping PE fed while
    the scalar engine drains exps and evictions.
  - PSUM accumulation groups never interleave within a bank (the
    accumulate-zero region is the whole bank).
"""

import numpy as np

import concourse.mybir as mybir
import concourse.tile as tile
from concourse import bacc
from concourse.bass_utils import run_bass_kernel_spmd

P = 128
B = 4
S = 2048
D = 1024
ND = D // P          # 128-chunks along any d/e/f/g axis (8)
NQ = 8               # query slots per core (128 rows each)
NT = S // P          # 128-row key tiles in the full sequence (16)
F32 = mybir.dt.float32
BF16 = mybir.dt.bfloat16

MASK_VAL = -1.0e5    # additive pre-scale mask; exp((s+MASK_VAL)/32) == 0.0

_CACHE: dict = {}


def build_program(reps: int = 1):
    """Single SPMD Bass program (same instruction stream on all 8 cores;
    per-core variation lives in the input data).  reps>1 repeats the
    body serially (timing-measurement variants)."""
    nc = bacc.Bacc(None)

    wq = nc.dram_tensor("wq", [D, D], BF16, kind="ExternalInput")
    wk = nc.dram_tensor("wk", [D, D], BF16, kind="ExternalInput")
    wvT = nc.dram_tensor("wvT", [D, D], BF16, kind="ExternalInput")
    xq = nc.dram_tensor("xq", [D, NQ * P], BF16, kind="ExternalInput")
    xT = nc.dram_tensor("xT", [D, S], BF16, kind="ExternalInput")
    xn = nc.dram_tensor("xn", [S, D], BF16, kind="ExternalInput")
    mask = nc.dram_tensor("mask", [NQ, 2, P, P], BF16, kind="ExternalInput")
    ones = nc.dram_tensor("ones", [P, 1], BF16, kind="ExternalInput")
    out = nc.dram_tensor("out", [NQ * P, D], F32, kind="ExternalOutput")

    wq_r = wq[:].rearrange("(i p) f -> p i f", p=P)
    wk_r = wk[:].rearrange("(i p) g -> p i g", p=P)
    wvT_r = wvT[:].rearrange("(i p) e -> p i e", p=P)
    xq_r = xq[:].rearrange("(i p) q -> p i q", p=P)
    xT_r = xT[:].rearrange("(i p) k -> p i k", p=P)
    xn_r = xn[:].rearrange("(t p) d -> p t d", p=P)

    with tile.TileContext(nc) as tc:
      for _rep in range(reps):
        with (
            tc.tile_pool(name="big", bufs=1) as bigp,
            tc.tile_pool(name="et", bufs=17) as etp,
            tc.tile_pool(name="stat", bufs=2) as statp,
            tc.tile_pool(name="axt", bufs=2) as axtp,
            tc.tile_pool(name="orow", bufs=2) as orowp,
        ):
            xT_s = bigp.tile([P, ND, S], BF16, tag="xT")
            xn_s = bigp.tile([P, NT, D], BF16, tag="xn")
            wvT_s = bigp.tile([P, ND, D], BF16, tag="wvT")
            G_s = bigp.tile([P, ND, NQ * P], BF16, tag="G")
            mask_s = bigp.tile([P, NQ, 2, P], BF16, tag="mask")
            ones_s = bigp.tile([P, 1], BF16, tag="ones")

            def score_tile(psp, j, kt, nt, ets, tag="pst", bufs=None):
                # scoresT[k, q] for key tile kt: x^T tiles stationary,
                # G tile moving; exp lands straight in A^T layout
                pst = psp.tile([P, P], F32, tag=tag, bufs=bufs)
                for gc in range(ND):
                    nc.tensor.matmul(
                        pst[:],
                        xT_s[:, gc, kt * P : (kt + 1) * P],
                        G_s[:, gc, j * P : (j + 1) * P],
                        start=(gc == 0),
                        stop=(gc == ND - 1),
                    )
                if kt >= nt - 2:
                    # causal mask data on the two diagonal-pair tiles
                    nc.vector.tensor_add(
                        pst[:], pst[:], mask_s[:, j, kt - (nt - 2), :]
                    )
                et = etp.tile([P, P], BF16, tag="et")
                nc.scalar.activation(
                    et[:],
                    pst[:],
                    mybir.ActivationFunctionType.Exp,
                    scale=float(1.0 / np.sqrt(D)),
                )
                ets.append(et)

            ets0 = []
            rcp0 = statp.tile([P, 1], F32, tag="rcp", name="rcp0")

            # ---- phase W/G: W_eff = Wq^T Wk, then G^T = W_eff^T x_q^T ----
            with (
                tc.tile_pool(name="wph", bufs=1) as wp,
                tc.tile_pool(name="ps_w", bufs=6, space="PSUM") as pswp,
            ):
                wq_s = wp.tile([P, ND, D], BF16, tag="wq")
                wk_s = wp.tile([P, ND, D], BF16, tag="wk")
                W_s = wp.tile([P, ND, D], BF16, tag="W")
                xq_s = wp.tile([P, ND, NQ * P], BF16, tag="xq")

                # wq/wk interleaved at fine granularity so the ec-stepped
                # W_eff chains below start as soon as the first slices
                # land; wk split by g-half (first chains read cols 0:512)
                for i in range(2):
                    nc.sync.dma_start(wq_s[:, i : i + 1, :], wq_r[:, i : i + 1, :])
                    nc.sync.dma_start(
                        wk_s[:, i : i + 1, 0:512], wk_r[:, i : i + 1, 0:512]
                    )
                for i in range(2, ND, 2):
                    nc.sync.dma_start(wq_s[:, i : i + 2, :], wq_r[:, i : i + 2, :])
                    nc.sync.dma_start(
                        wk_s[:, i : i + 2, 0:512], wk_r[:, i : i + 2, 0:512]
                    )
                for i in range(0, ND, 2):
                    nc.sync.dma_start(
                        wk_s[:, i : i + 2, 512:D], wk_r[:, i : i + 2, 512:D]
                    )
                for i in range(0, ND, 2):
                    nc.sync.dma_start(xq_s[:, i : i + 2, :], xq_r[:, i : i + 2, :])
                for i in range(0, ND, 2):
                    nc.sync.dma_start(xT_s[:, i : i + 2, :], xT_r[:, i : i + 2, :])
                for t in range(0, NT, 4):
                    nc.sync.dma_start(xn_s[:, t : t + 4, :], xn_r[:, t : t + 4, :])
                for i in range(0, ND, 2):
                    nc.sync.dma_start(wvT_s[:, i : i + 2, :], wvT_r[:, i : i + 2, :])
                nc.sync.dma_start(mask_s[:], mask[:].rearrange("j i p q -> p j i q"))
                nc.sync.dma_start(ones_s[:], ones[:])

                # W_eff[f, g] = sum_e Wq[e, f] Wk[e, g]   (f in partitions)
                def w_chain(gh, fc, pw):
                    for ec in range(ND):
                        nc.tensor.matmul(
                            pw[:],
                            wq_s[:, ec, fc * P : (fc + 1) * P],
                            wk_s[:, ec, gh * 512 : (gh + 1) * 512],
                            start=(ec == 0),
                            stop=(ec == ND - 1),
                        )

                # window of 6 ec-stepped chains overlapping the weight DMA
                pws = [
                    pswp.tile([P, 512], F32, tag="pw", name=f"pw0_{fc}")
                    for fc in range(6)
                ]
                for ec in range(ND):
                    for fc in range(6):
                        nc.tensor.matmul(
                            pws[fc][:],
                            wq_s[:, ec, fc * P : (fc + 1) * P],
                            wk_s[:, ec, 0:512],
                            start=(ec == 0),
                            stop=(ec == ND - 1),
                        )
                for fc in range(6):
                    nc.scalar.copy(W_s[:, fc, 0:512], pws[fc][:])
                for gh, fc in [(0, 6), (0, 7)] + [(1, fc) for fc in range(ND)]:
                    pw = pswp.tile([P, 512], F32, tag="pw", name=f"pw{gh}_{fc}")
                    w_chain(gh, fc, pw)
                    nc.scalar.copy(W_s[:, fc, gh * 512 : (gh + 1) * 512], pw[:])

                # G^T[g, q] = sum_f W_eff[f, g] x_q^T[f, q]  (g in partitions)
                # qh-outer: scores of slot 0 need q-columns 0:128 for all gc
                def g_chain(qh, gc):
                    pg = pswp.tile([P, 512], F32, tag="pw", name=f"pg{qh}_{gc}")
                    for fc in range(ND):
                        nc.tensor.matmul(
                            pg[:],
                            W_s[:, fc, gc * P : (gc + 1) * P],
                            xq_s[:, fc, qh * 512 : (qh + 1) * 512],
                            start=(fc == 0),
                            stop=(fc == ND - 1),
                        )
                    nc.scalar.copy(G_s[:, gc, qh * 512 : (qh + 1) * 512], pg[:])

                for gc in range(ND):
                    g_chain(0, gc)
                for gc in range(ND - 1):
                    g_chain(1, gc)
                # slot 0's two score tiles on the spare PSUM banks; their
                # mask/exp drain while the last G chain computes
                score_tile(pswp, 0, 0, 2, ets0, tag="pst00", bufs=1)
                score_tile(pswp, 0, 1, 2, ets0, tag="pst01", bufs=1)
                g_chain(1, ND - 1)

            # ---- phase A: attention + output projection ----
            with (
                tc.tile_pool(name="ps_s", bufs=4, space="PSUM") as pssp,
                tc.tile_pool(name="ps_d", bufs=2, space="PSUM") as psdp,
                tc.tile_pool(name="ps_a", bufs=2, space="PSUM") as psap,
            ):

                def proj_flush(axt, rcp, j):
                    # out = (AX) @ Wv^T, normalized by 1/den at eviction
                    orow = orowp.tile([P, D], F32, tag="orow")
                    for eh in range(2):
                        po = psap.tile([P, 512], F32, tag="pav", name=f"po{j}_{eh}")
                        for dc in range(ND):
                            nc.tensor.matmul(
                                po[:],
                                axt[:, dc * P : (dc + 1) * P],
                                wvT_s[:, dc, eh * 512 : (eh + 1) * 512],
                                start=(dc == 0),
                                stop=(dc == ND - 1),
                            )
                        nc.vector.tensor_scalar_mul(
                            orow[:, eh * 512 : (eh + 1) * 512], po[:], rcp[:]
                        )
                        nc.sync.dma_start(
                            out[j * P : (j + 1) * P, eh * 512 : (eh + 1) * 512],
                            orow[:, eh * 512 : (eh + 1) * 512],
                        )

                def axt_group(paxs, ets, dc, nt):
                    # AX^T[d, q] for one 128-wide d-chunk: x key tiles
                    # stationary, exp-score tiles moving.  One PSUM
                    # accumulation group at a time per bank (the zero
                    # region is the bank, groups must not interleave).
                    sub = dc % 4
                    for kt in range(nt):
                        nc.tensor.matmul(
                            paxs[dc // 4][:, sub * P : (sub + 1) * P],
                            xn_s[:, kt, dc * P : (dc + 1) * P],
                            ets[kt][:],
                            start=(kt == 0),
                            stop=(kt == nt - 1),
                        )

                def den_rcp(ets, rcp, nt, j):
                    # den[q] = sum_k A^T[k, q] via 1-column matmuls
                    pden = psdp.tile([P, 1], F32, tag="pden", name=f"pden{j}")
                    for kt in range(nt):
                        nc.tensor.matmul(
                            pden[:],
                            ets[kt][:],
                            ones_s[:],
                            start=(kt == 0),
                            stop=(kt == nt - 1),
                        )
                    nc.vector.reciprocal(rcp[:], pden[:])

                pending = None
                for j in range(NQ):
                    nt = 2 * (j + 1)    # 128-wide key tiles in the span
                    if j == 0:
                        ets, rcp = ets0, rcp0
                    else:
                        rcp = statp.tile([P, 1], F32, tag="rcp")
                        ets = []
                        for kt in range(nt):
                            score_tile(pssp, j, kt, nt, ets)
                        if pending is not None:
                            proj_flush(*pending)

                    # AX^T accumulation, one 128-wide d-chunk per group;
                    # evict each half as soon as its groups complete
                    paxs = [
                        psap.tile([P, 512], F32, tag="pav", name=f"pax{j}_{dh}")
                        for dh in range(2)
                    ]
                    axt = axtp.tile([P, D], BF16, tag="axt")
                    for dc in range(ND):
                        axt_group(paxs, ets, dc, nt)
                        if dc % 4 == 3:
                            dh = dc // 4
                            nc.scalar.copy(
                                axt[:, dh * 512 : (dh + 1) * 512], paxs[dh][:]
                            )
                    den_rcp(ets, rcp, nt, j)
                    pending = (axt, rcp, j)

                proj_flush(*pending)

    nc.finalize()
    return nc


def make_mask(h: int) -> np.ndarray:
    """Additive masks for the two diagonal-pair key tiles of each slot,
    in transposed [key, query] layout."""
    import ml_dtypes

    m = np.zeros((NQ, 2, P, P), dtype=ml_dtypes.bfloat16)
    k_r = np.arange(P)[:, None]
    q_r = np.arange(P)[None, :]
    triT = np.where(q_r >= k_r, 0.0, MASK_VAL).astype(ml_dtypes.bfloat16)
    for j in range(NQ):
        if h == 1:
            # q-tile 2j+1: key tile 2j fully valid, diagonal in 2j+1
            m[j, 1] = triT
        else:
            # q-tile 2j: diagonal in key tile 2j, tile 2j+1 fully masked
            m[j, 0] = triT
            m[j, 1] = MASK_VAL
    return m


def make_in_maps(x, Wq, Wk, Wv):
    import ml_dtypes

    bf16 = ml_dtypes.bfloat16
    x = np.asarray(x, dtype=np.float32)
    wq_b = np.ascontiguousarray(np.asarray(Wq, dtype=np.float32).astype(bf16))
    wk_b = np.ascontiguousarray(np.asarray(Wk, dtype=np.float32).astype(bf16))
    wvT_b = np.ascontiguousarray(np.asarray(Wv, dtype=np.float32).T.astype(bf16))
    ones = np.ones((P, 1), dtype=bf16)
    masks = [make_mask(0), make_mask(1)]
    in_maps = []
    for c in range(8):
        b, h = c // 2, c % 2
        xb = x[b].astype(bf16)                                  # [S, D]
        xT_b = np.ascontiguousarray(xb.T)                       # [D, S]
        xq_b = np.ascontiguousarray(
            xT_b.reshape(D, NT, P)[:, [2 * j + h for j in range(NQ)], :].reshape(
                D, NQ * P
            )
        )
        in_maps.append(
            {
                "wq": wq_b,
                "wk": wk_b,
                "wvT": wvT_b,
                "xq": xq_b,
                "xT": xT_b,
                "xn": xb,
                "mask": masks[h],
                "ones": ones,
            }
        )
    return in_maps


def gather_output(results) -> np.ndarray:
    out = np.empty((B, S, D), dtype=np.float32)
    for c in range(8):
        b, h = c // 2, c % 2
        oc = results[c]["out"]
        for j in range(NQ):
            t = 2 * j + h
            out[b, t * P : (t + 1) * P, :] = oc[j * P : (j + 1) * P, :]
    return out


def kernel(x, Wq, Wk, Wv):
    if "p1" not in _CACHE:
        _CACHE["p1"] = build_program()
    nc = _CACHE["p1"]
    in_maps = make_in_maps(x, Wq, Wk, Wv)
    res = run_bass_kernel_spmd(nc, in_maps, core_ids=list(range(8)))
    return gather_output(res.results)


# revision 18
# speedup vs baseline: 1.6598x; 1.1168x over previous
"""Causal single-head attention on 8 Trainium2 NeuronCores.

Problem: x[4, 2048, 1024], Wq/Wk/Wv[1024, 1024] (torch Linear layout).
  q = x @ Wq.T ; k = x @ Wk.T ; v = x @ Wv.T
  out = softmax(mask(q @ k.T) / 32) @ v

Sharding: 8 cores = (batch b = core // 2) x (query-parity h = core % 2).
Parity interleaving (q-tiles t = 2j + h) makes the per-slot causal span
structure identical across cores, so a single SPMD program serves all 8.

Algebraic restructure vs the direct form: the K and V projections of the
full sequence would be duplicated on both cores of a batch (the dominant
cost).  Instead
  scores = q @ k.T = x @ (Wq.T @ Wk) @ x.T  =: (x_q @ W_eff) @ x.T
  out    = A @ v   = (A @ x) @ Wv.T
so the full-sequence operand of both attention matmuls is the *raw
input* x (no projection), and the per-core projection work is W_eff
(one 1024^3 gemm), G = x_q @ W_eff (own queries only), and the final
(A @ x) @ Wv.T (own queries only).  Per-core tensor cycles drop from
~493k to ~344k.

All matmul operands are bf16 (1 cycle/row on TRN2 regardless of moving
width; fp32 PSUM accumulation; end-to-end rel err ~4e-3 vs the 2e-2
gate).  Scores are computed directly transposed ([key, query] tiles,
x^T tiles stationary / G tiles moving), so exp writes the A^T operand
of the A@x matmul in place -- no transpose pass.  The softmax
denominator comes from near-free 1-column matmuls den = A^T.T @ ones;
the max-subtract is skipped (logits are O(1) after the 1/32 scale) and
1/den is folded into the final eviction of the output row.

Scheduling notes (PE bubbles cost double: the clock drops to 1.2 GHz
for 3 us after any idle gap):
  - W_eff starts as 6 concurrent PSUM chains stepped by contraction
    chunk, so compute starts as soon as the first wq/wk slices land
    instead of waiting for the full weight load; wk loads its g-halves
    separately since the first chains only read columns 0:512.
  - Slot 0's score tiles are computed between the last two G chains
    (on spare PSUM banks) so their exps hide under G compute and the
    PSUM pool transition.
  - Each slot's output projection is emitted *between* the score tiles
    and the A@x accumulation of the next slot, keeping PE fed while
    the scalar engine drains exps and evictions.
  - PSUM accumulation groups never interleave within a bank (the
    accumulate-zero region is the whole bank).
"""

import numpy as np

import concourse.mybir as mybir
import concourse.tile as tile
from concourse import bacc
from concourse.bass_utils import run_bass_kernel_spmd

P = 128
B = 4
S = 2048
D = 1024
ND = D // P          # 128-chunks along any d/e/f/g axis (8)
NQ = 8               # query slots per core (128 rows each)
NT = S // P          # 128-row key tiles in the full sequence (16)
F32 = mybir.dt.float32
BF16 = mybir.dt.bfloat16

MASK_VAL = -1.0e5    # additive pre-scale mask; exp((s+MASK_VAL)/32) == 0.0

_CACHE: dict = {}


def build_program(reps: int = 1):
    """Single SPMD Bass program (same instruction stream on all 8 cores;
    per-core variation lives in the input data).  reps>1 repeats the
    body serially (timing-measurement variants)."""
    nc = bacc.Bacc(None)

    wq = nc.dram_tensor("wq", [D, D], BF16, kind="ExternalInput")
    wk = nc.dram_tensor("wk", [D, D], BF16, kind="ExternalInput")
    wvT = nc.dram_tensor("wvT", [D, D], BF16, kind="ExternalInput")
    xq = nc.dram_tensor("xq", [D, NQ * P], BF16, kind="ExternalInput")
    xT = nc.dram_tensor("xT", [D, S], BF16, kind="ExternalInput")
    xn = nc.dram_tensor("xn", [S, D], BF16, kind="ExternalInput")
    mask = nc.dram_tensor("mask", [NQ, 2, P, P], BF16, kind="ExternalInput")
    ones = nc.dram_tensor("ones", [P, 1], BF16, kind="ExternalInput")
    out = nc.dram_tensor("out", [NQ * P, D], F32, kind="ExternalOutput")

    wq_r = wq[:].rearrange("(i p) f -> p i f", p=P)
    wk_r = wk[:].rearrange("(i p) g -> p i g", p=P)
    wvT_r = wvT[:].rearrange("(i p) e -> p i e", p=P)
    xq_r = xq[:].rearrange("(i p) q -> p i q", p=P)
    xT_r = xT[:].rearrange("(i p) k -> p i k", p=P)
    xn_r = xn[:].rearrange("(t p) d -> p t d", p=P)

    with tile.TileContext(nc) as tc:
      for _rep in range(reps):
        with (
            tc.tile_pool(name="big", bufs=1) as bigp,
            tc.tile_pool(name="et", bufs=17) as etp,
            tc.tile_pool(name="stat", bufs=2) as statp,
            tc.tile_pool(name="axt", bufs=2) as axtp,
            tc.tile_pool(name="orow", bufs=2) as orowp,
        ):
            xT_s = bigp.tile([P, ND, S], BF16, tag="xT")
            xn_s = bigp.tile([P, NT, D], BF16, tag="xn")
            wvT_s = bigp.tile([P, ND, D], BF16, tag="wvT")
            G_s = bigp.tile([P, ND, NQ * P], BF16, tag="G")
            mask_s = bigp.tile([P, NQ, 2, P], BF16, tag="mask")
            ones_s = bigp.tile([P, 1], BF16, tag="ones")

            def score_tile(psp, j, kt, nt, ets, tag="pst", bufs=None):
                # scoresT[k, q] for key tile kt: x^T tiles stationary,
                # G tile moving; exp lands straight in A^T layout
                pst = psp.tile([P, P], F32, tag=tag, bufs=bufs)
                for gc in range(ND):
                    nc.tensor.matmul(
                        pst[:],
                        xT_s[:, gc, kt * P : (kt + 1) * P],
                        G_s[:, gc, j * P : (j + 1) * P],
                        start=(gc == 0),
                        stop=(gc == ND - 1),
                    )
                if kt >= nt - 2:
                    # causal mask data on the two diagonal-pair tiles
                    nc.vector.tensor_add(
                        pst[:], pst[:], mask_s[:, j, kt - (nt - 2), :]
                    )
                et = etp.tile([P, P], BF16, tag="et")
                nc.scalar.activation(
                    et[:],
                    pst[:],
                    mybir.ActivationFunctionType.Exp,
                    scale=float(1.0 / np.sqrt(D)),
                )
                ets.append(et)

            ets0 = []
            rcp0 = statp.tile([P, 1], F32, tag="rcp", name="rcp0")

            # ---- phase W/G: W_eff = Wq^T Wk, then G^T = W_eff^T x_q^T ----
            with (
                tc.tile_pool(name="wph", bufs=1) as wp,
                tc.tile_pool(name="ps_w", bufs=6, space="PSUM") as pswp,
            ):
                wq_s = wp.tile([P, ND, D], BF16, tag="wq")
                wk_s = wp.tile([P, ND, D], BF16, tag="wk")
                W_s = wp.tile([P, ND, D], BF16, tag="W")
                xq_s = wp.tile([P, ND, NQ * P], BF16, tag="xq")

                # wq/wk interleaved at fine granularity so the ec-stepped
                # W_eff chains below start as soon as the first slices
                # land; wk split by g-half (first chains read cols 0:512)
                # first slices small so chain (ec0, fc0) starts earliest
                nc.sync.dma_start(wq_s[:, 0:1, 0:P], wq_r[:, 0:1, 0:P])
                nc.sync.dma_start(wk_s[:, 0:1, 0:512], wk_r[:, 0:1, 0:512])
                nc.sync.dma_start(wq_s[:, 0:1, P:D], wq_r[:, 0:1, P:D])
                nc.sync.dma_start(wq_s[:, 1:2, :], wq_r[:, 1:2, :])
                nc.sync.dma_start(wk_s[:, 1:2, 0:512], wk_r[:, 1:2, 0:512])
                for i in range(2, ND, 2):
                    nc.sync.dma_start(wq_s[:, i : i + 2, :], wq_r[:, i : i + 2, :])
                    nc.sync.dma_start(
                        wk_s[:, i : i + 2, 0:512], wk_r[:, i : i + 2, 0:512]
                    )
                for i in range(0, ND, 2):
                    nc.sync.dma_start(
                        wk_s[:, i : i + 2, 512:D], wk_r[:, i : i + 2, 512:D]
                    )
                for i in range(0, ND, 2):
                    nc.sync.dma_start(xq_s[:, i : i + 2, :], xq_r[:, i : i + 2, :])
                for i in range(0, ND, 2):
                    nc.sync.dma_start(xT_s[:, i : i + 2, :], xT_r[:, i : i + 2, :])
                for t in range(0, NT, 4):
                    nc.sync.dma_start(xn_s[:, t : t + 4, :], xn_r[:, t : t + 4, :])
                for i in range(0, ND, 2):
                    nc.sync.dma_start(wvT_s[:, i : i + 2, :], wvT_r[:, i : i + 2, :])
                nc.sync.dma_start(mask_s[:], mask[:].rearrange("j i p q -> p j i q"))
                nc.sync.dma_start(ones_s[:], ones[:])

                # W_eff[f, g] = sum_e Wq[e, f] Wk[e, g]   (f in partitions)
                def w_chain(gh, fc, pw):
                    for ec in range(ND):
                        nc.tensor.matmul(
                            pw[:],
                            wq_s[:, ec, fc * P : (fc + 1) * P],
                            wk_s[:, ec, gh * 512 : (gh + 1) * 512],
                            start=(ec == 0),
                            stop=(ec == ND - 1),
                        )

                # window of 6 ec-stepped chains overlapping the weight DMA
                pws = [
                    pswp.tile([P, 512], F32, tag="pw", name=f"pw0_{fc}")
                    for fc in range(6)
                ]
                for ec in range(ND):
                    for fc in range(6):
                        nc.tensor.matmul(
                            pws[fc][:],
                            wq_s[:, ec, fc * P : (fc + 1) * P],
                            wk_s[:, ec, 0:512],
                            start=(ec == 0),
                            stop=(ec == ND - 1),
                        )
                for fc in range(6):
                    nc.scalar.copy(W_s[:, fc, 0:512], pws[fc][:])
                for gh, fc in [(0, 6), (0, 7)] + [(1, fc) for fc in range(ND)]:
                    pw = pswp.tile([P, 512], F32, tag="pw", name=f"pw{gh}_{fc}")
                    w_chain(gh, fc, pw)
                    nc.scalar.copy(W_s[:, fc, gh * 512 : (gh + 1) * 512], pw[:])

                # G^T[g, q] = sum_f W_eff[f, g] x_q^T[f, q]  (g in partitions)
                # qh-outer: scores of slot 0 need q-columns 0:128 for all gc
                def g_chain(qh, gc, split_evict=False):
                    pg = pswp.tile([P, 512], F32, tag="pw", name=f"pg{qh}_{gc}")
                    for fc in range(ND):
                        nc.tensor.matmul(
                            pg[:],
                            W_s[:, fc, gc * P : (gc + 1) * P],
                            xq_s[:, fc, qh * 512 : (qh + 1) * 512],
                            start=(fc == 0),
                            stop=(fc == ND - 1),
                        )
                    base = qh * 512
                    if split_evict:
                        # last ps_w reader gates the PSUM pool transition:
                        # halve its latency by splitting across ACT and DVE
                        nc.scalar.copy(
                            G_s[:, gc, base : base + 256], pg[:, 0:256]
                        )
                        nc.vector.tensor_copy(
                            G_s[:, gc, base + 256 : base + 512], pg[:, 256:512]
                        )
                    else:
                        nc.scalar.copy(G_s[:, gc, base : base + 512], pg[:])

                for gc in range(ND):
                    g_chain(0, gc)
                for gc in range(ND - 1):
                    g_chain(1, gc)
                # slot 0's two score tiles on the spare PSUM banks; their
                # mask/exp drain while the last G chain computes
                score_tile(pswp, 0, 0, 2, ets0, tag="pst00", bufs=1)
                score_tile(pswp, 0, 1, 2, ets0, tag="pst01", bufs=1)
                g_chain(1, ND - 1, split_evict=True)

            # ---- phase A: attention + output projection ----
            with (
                tc.tile_pool(name="ps_s", bufs=4, space="PSUM") as pssp,
                tc.tile_pool(name="ps_d", bufs=2, space="PSUM") as psdp,
                tc.tile_pool(name="ps_a", bufs=2, space="PSUM") as psap,
            ):

                def proj_flush(axt, rcp, j):
                    # out = (AX) @ Wv^T, normalized by 1/den at eviction.
                    # The last slot pipelines evict+DMA in quarters since
                    # nothing else hides its tail.
                    pieces = 1 if j < NQ - 1 else 2
                    orow = orowp.tile([P, D], F32, tag="orow")
                    for eh in range(2):
                        po = psap.tile([P, 512], F32, tag="pav", name=f"po{j}_{eh}")
                        for dc in range(ND):
                            nc.tensor.matmul(
                                po[:],
                                axt[:, dc * P : (dc + 1) * P],
                                wvT_s[:, dc, eh * 512 : (eh + 1) * 512],
                                start=(dc == 0),
                                stop=(dc == ND - 1),
                            )
                        w = 512 // pieces
                        for pc in range(pieces):
                            base = eh * 512 + pc * w
                            nc.vector.tensor_scalar_mul(
                                orow[:, base : base + w], po[:, pc * w : pc * w + w],
                                rcp[:],
                            )
                            nc.sync.dma_start(
                                out[j * P : (j + 1) * P, base : base + w],
                                orow[:, base : base + w],
                            )

                def axt_group(paxs, ets, dc, nt):
                    # AX^T[d, q] for one 128-wide d-chunk: x key tiles
                    # stationary, exp-score tiles moving.  One PSUM
                    # accumulation group at a time per bank (the zero
                    # region is the bank, groups must not interleave).
                    sub = dc % 4
                    for kt in range(nt):
                        nc.tensor.matmul(
                            paxs[dc // 4][:, sub * P : (sub + 1) * P],
                            xn_s[:, kt, dc * P : (dc + 1) * P],
                            ets[kt][:],
                            start=(kt == 0),
                            stop=(kt == nt - 1),
                        )

                def den_rcp(ets, rcp, nt, j):
                    # den[q] = sum_k A^T[k, q] via 1-column matmuls
                    pden = psdp.tile([P, 1], F32, tag="pden", name=f"pden{j}")
                    for kt in range(nt):
                        nc.tensor.matmul(
                            pden[:],
                            ets[kt][:],
                            ones_s[:],
                            start=(kt == 0),
                            stop=(kt == nt - 1),
                        )
                    nc.vector.reciprocal(rcp[:], pden[:])

                pending = None
                for j in range(NQ):
                    nt = 2 * (j + 1)    # 128-wide key tiles in the span
                    if j == 0:
                        ets, rcp = ets0, rcp0
                    else:
                        rcp = statp.tile([P, 1], F32, tag="rcp")
                        ets = []
                        for kt in range(nt):
                            score_tile(pssp, j, kt, nt, ets)
                        if pending is not None:
                            proj_flush(*pending)

                    # AX^T accumulation, one 128-wide d-chunk per group;
                    # evict each half as soon as its groups complete
                    paxs = [
                        psap.tile([P, 512], F32, tag="pav", name=f"pax{j}_{dh}")
                        for dh in range(2)
                    ]
                    axt = axtp.tile([P, D], BF16, tag="axt")
                    for dc in range(ND):
                        axt_group(paxs, ets, dc, nt)
                        if dc % 4 == 3:
                            dh = dc // 4
                            nc.scalar.copy(
                                axt[:, dh * 512 : (dh + 1) * 512], paxs[dh][:]
                            )
                    den_rcp(ets, rcp, nt, j)
                    pending = (axt, rcp, j)

                proj_flush(*pending)

    nc.finalize()
    return nc


def make_mask(h: int) -> np.ndarray:
    """Additive masks for the two diagonal-pair key tiles of each slot,
    in transposed [key, query] layout."""
    import ml_dtypes

    m = np.zeros((NQ, 2, P, P), dtype=ml_dtypes.bfloat16)
    k_r = np.arange(P)[:, None]
    q_r = np.arange(P)[None, :]
    triT = np.where(q_r >= k_r, 0.0, MASK_VAL).astype(ml_dtypes.bfloat16)
    for j in range(NQ):
        if h == 1:
            # q-tile 2j+1: key tile 2j fully valid, diagonal in 2j+1
            m[j, 1] = triT
        else:
            # q-tile 2j: diagonal in key tile 2j, tile 2j+1 fully masked
            m[j, 0] = triT
            m[j, 1] = MASK_VAL
    return m


def make_in_maps(x, Wq, Wk, Wv):
    import ml_dtypes

    bf16 = ml_dtypes.bfloat16
    x = np.asarray(x, dtype=np.float32)
    wq_b = np.ascontiguousarray(np.asarray(Wq, dtype=np.float32).astype(bf16))
    wk_b = np.ascontiguousarray(np.asarray(Wk, dtype=np.float32).astype(bf16))
    wvT_b = np.ascontiguousarray(np.asarray(Wv, dtype=np.float32).T.astype(bf16))
    ones = np.ones((P, 1), dtype=bf16)
    masks = [make_mask(0), make_mask(1)]
    in_maps = []
    for c in range(8):
        b, h = c // 2, c % 2
        xb = x[b].astype(bf16)                                  # [S, D]
        xT_b = np.ascontiguousarray(xb.T)                       # [D, S]
        xq_b = np.ascontiguousarray(
            xT_b.reshape(D, NT, P)[:, [2 * j + h for j in range(NQ)], :].reshape(
                D, NQ * P
            )
        )
        in_maps.append(
            {
                "wq": wq_b,
                "wk": wk_b,
                "wvT": wvT_b,
                "xq": xq_b,
                "xT": xT_b,
                "xn": xb,
                "mask": masks[h],
                "ones": ones,
            }
        )
    return in_maps


def gather_output(results) -> np.ndarray:
    out = np.empty((B, S, D), dtype=np.float32)
    for c in range(8):
        b, h = c // 2, c % 2
        oc = results[c]["out"]
        for j in range(NQ):
            t = 2 * j + h
            out[b, t * P : (t + 1) * P, :] = oc[j * P : (j + 1) * P, :]
    return out


def kernel(x, Wq, Wk, Wv):
    if "p1" not in _CACHE:
        _CACHE["p1"] = build_program()
    nc = _CACHE["p1"]
    in_maps = make_in_maps(x, Wq, Wk, Wv)
    res = run_bass_kernel_spmd(nc, in_maps, core_ids=list(range(8)))
    return gather_output(res.results)
